# revision 14
# baseline (speedup 1.0000x reference)
import os
"""2-layer GAT (100k nodes, 1.7M edges incl self-loops).

Default path: the 8-core TRN2 Bass pipeline below (rel err ~3e-3, bf16
records). The historical dma_gather NRT_EXEC_UNIT_UNRECOVERABLE crash was
swdge descriptor-scratch exhaustion across the two edge phases; fixed by
dynamic_dma_scratch_size=49152 + num_swdge_queues=2 with layer 2's gathers
on queue 1. Any device failure falls back to the exact host numpy path.

Node-parallel sharding: each core owns a 12500-node dst range. Edges are
routed to the dst owner, dst-sorted, grouped by aligned 128-node windows,
and split into 4 source-quarter passes so dma_gather's int16 indices can
address the padded global record table (25344 rows per quarter).

Per layer: dense projection + attention logits on PE -> per-node 256B bf16
records -> AllGather -> edge phase (dma_gather records by src, one-hot
scatter matmuls accumulating per-window segment sums in PSUM, al_dst
expanded per edge via PE transpose of the one-hot) -> segment-softmax
normalize -> relu/bias + next projection (L1) or log_softmax (L2).
"""
import numpy as np
import ml_dtypes

N = 100000
IN_DIM = 512
HEADS = 8
CH = 8
F1 = 64
OUT_DIM = 64
NEG = 0.2
NCORES = 8
EPS = 1e-9
bf16 = ml_dtypes.bfloat16

DLOC = N // NCORES            # dst nodes per core
PW = 128                      # window node count
NW = (DLOC + PW - 1) // PW    # real windows per core
NL = (NW + 1) * 128           # local table rows (+1 dummy block)
QROWS = 2 * NL                # rows per source quarter (2 cores)
REC = 128                     # record width in bf16 elements (256B)
MAXCH = 40                    # max chunks per gather group
GCAP = 8                      # max chunks (1024 idx) per dma_gather instr

LAST_RESULTS = None


def _prep_edges(src, dst):
    """Per-core edge structures shared by both layers (same graph)."""
    order = np.argsort(dst, kind="stable")
    src = src[order].astype(np.int64)
    dst = dst[order].astype(np.int64)
    core_of = dst // DLOC
    grow = (src // DLOC) * NL + (src % DLOC)
    quarter = grow // QROWS

    counts = np.zeros((NCORES, 4, NW), np.int64)
    per = []
    for i in range(NCORES):
        m = core_of == i
        dl = dst[m] - i * DLOC
        q = quarter[m]
        per.append((q, dl // PW, grow[m] - q * QROWS, dl % PW))
        for qq in range(4):
            counts[i, qq] = np.bincount(per[i][1][q == qq], minlength=NW)

    K = np.ceil(counts.max(axis=0) / 128.0).astype(np.int64)  # [4, NW]

    groups = []  # (q, w0, w1, chunk_off, nch)
    nch_total = 0
    for q in range(4):
        w0 = 0
        while w0 < NW:
            w1, c = w0, 0
            while w1 < NW and c + K[q, w1] <= MAXCH:
                c += int(K[q, w1])
                w1 += 1
            if w1 == w0:
                w1, c = w0 + 1, int(K[q, w0])
            if c > 0:
                groups.append((q, w0, w1, nch_total, c))
                nch_total += c
            else:
                groups.append((q, w0, w1, nch_total, 0))
            w0 = w1
    groups = [g for g in groups if g[4] > 0]

    # slotbase[q, w]: flat slot (in 128-row chunks) where window w's edge
    # entries start within the concatenated group blocks
    slotbase = np.zeros((4, NW), np.int64)
    for (qq, w0, w1, coff, nch) in groups:
        k0 = coff
        for w in range(w0, w1):
            slotbase[qq, w] = k0
            k0 += int(K[qq, w])
        assert k0 == coff + nch
    dummy = NW * 128
    qidx = np.zeros((NCORES, 128, nch_total * 8), np.int16)
    dstpos = np.full((NCORES, 128, nch_total), 127, np.int16)
    for i in range(NCORES):
        q_, w_, g_, pos_ = per[i]
        ilist = np.full(nch_total * 128, dummy, np.int64)
        plist = np.full(nch_total * 128, 127, np.int64)
        for qq in range(4):
            mq = q_ == qq
            o = np.argsort(w_[mq], kind="stable")
            ww = w_[mq][o]
            bounds = np.searchsorted(ww, np.arange(NW + 1))
            j = np.arange(len(ww))
            slots = slotbase[qq, ww] * 128 + (j - bounds[ww])
            ilist[slots] = g_[mq][o]
            plist[slots] = pos_[mq][o]
        for (qq, w0, w1, coff, nch) in groups:
            blk = ilist[coff * 128:(coff + nch) * 128]
            wr = blk.reshape(-1, 16).T.astype(np.int16)
            qidx[i, :, coff * 8:(coff + nch) * 8] = np.tile(wr, (8, 1))
            dstpos[i, :, coff:coff + nch] = \
                plist[coff * 128:(coff + nch) * 128].reshape(nch, 128).T
    return groups, K, nch_total, qidx, dstpos


def _build_bass(groups, K, nch_total):
    import concourse.tile as tile
    from concourse import bacc, mybir
    from concourse.masks import make_identity

    dt = mybir.dt
    AF = mybir.ActivationFunctionType
    OP = mybir.AluOpType
    qidx_cols = nch_total * 8
    MAXCH_T = max(g[4] for g in groups)
    MAXK = int(K.max())
    NB = NL // 128

    nc = bacc.Bacc(None, target_bir_lowering=False, num_devices=NCORES,
                   dynamic_dma_scratch_size=49152, num_swdge_queues=2)
    xT_d = nc.dram_tensor("xT", [IN_DIM, NL], dt.float32, kind="ExternalInput")
    w1_d = nc.dram_tensor("w1", [IN_DIM, F1], dt.float32, kind="ExternalInput")
    a1_d = nc.dram_tensor("a1", [F1, 16], dt.bfloat16, kind="ExternalInput")
    w2_d = nc.dram_tensor("w2", [F1, OUT_DIM], dt.bfloat16, kind="ExternalInput")
    a2_d = nc.dram_tensor("a2", [OUT_DIM, 2], dt.bfloat16, kind="ExternalInput")
    b1_d = nc.dram_tensor("b1", [F1, 1], dt.float32, kind="ExternalInput")
    qidx_d = nc.dram_tensor("qidx", [128, qidx_cols], dt.int16, kind="ExternalInput")
    dpos_d = nc.dram_tensor("dpos", [128, nch_total], dt.bfloat16, kind="ExternalInput")
    out_d = nc.dram_tensor("out", [NL, OUT_DIM], dt.float32, kind="ExternalOutput")

    rloc1 = nc.dram_tensor("rloc1", [NL, REC], dt.bfloat16, kind="Internal")
    rglob1 = nc.dram_tensor("rglob1", [NCORES * NL, REC], dt.bfloat16,
                            kind="Internal", addr_space="Shared")
    adt1 = nc.dram_tensor("adt1", [NL, 8], dt.bfloat16, kind="Internal")
    rloc2 = nc.dram_tensor("rloc2", [NL, REC], dt.bfloat16, kind="Internal")
    rglob2 = nc.dram_tensor("rglob2", [NCORES * NL, REC], dt.bfloat16,
                            kind="Internal", addr_space="Shared")
    adt2 = nc.dram_tensor("adt2", [NL, 8], dt.bfloat16, kind="Internal")

    with tile.TileContext(nc) as tc:
        with (
            tc.tile_pool(name="const", bufs=1) as cpool,
            tc.tile_pool(name="dense", bufs=3) as dpool,
            tc.tile_pool(name="gath", bufs=2) as gpool,
            tc.tile_pool(name="edge", bufs=2) as epool,
            tc.tile_pool(name="s2p", bufs=3) as s2pool,
            tc.tile_pool(name="accp", bufs=1) as apool,
            tc.tile_pool(name="tailp", bufs=2) as tpool,
            tc.tile_pool(name="ps", bufs=2, space="PSUM") as ps,
        ):
            ident = cpool.tile([128, 128], dt.bfloat16)
            make_identity(nc, ident[:, :])
            iota_r = cpool.tile([128, 1, 128], dt.bfloat16)
            nc.gpsimd.iota(iota_r[:, :, :], pattern=[[0, 1], [1, 128]], base=0,
                           channel_multiplier=0,
                           allow_small_or_imprecise_dtypes=True)
            w1t = cpool.tile([128, 4, F1], dt.float32)
            nc.sync.dma_start(w1t[:, :, :],
                              w1_d[:, :].rearrange("(k p) f -> p k f", p=128))
            a1t = cpool.tile([F1, 16], dt.bfloat16)
            nc.sync.dma_start(a1t[:, :], a1_d[:, :])
            w2t = cpool.tile([F1, OUT_DIM], dt.bfloat16)
            nc.sync.dma_start(w2t[:, :], w2_d[:, :])
            a2t = cpool.tile([OUT_DIM, 2], dt.bfloat16)
            nc.sync.dma_start(a2t[:, :], a2_d[:, :])
            b1t = cpool.tile([F1, 1], dt.float32)
            nc.sync.dma_start(b1t[:, :], b1_d[:, :])
            m60 = cpool.tile([1, 8], dt.bfloat16)
            nc.vector.memset(m60[:, :], -60.0)
            dpt = cpool.tile([128, nch_total], dt.bfloat16)
            nc.sync.dma_start(dpt[:, :], dpos_d[:, :])
            qit = cpool.tile([128, qidx_cols], dt.int16)
            nc.sync.dma_start(qit[:, :], qidx_d[:, :])
            accs = apool.tile([128, NW, 72], dt.float32)

            # ---------------- dense layer 1
            for j in range(NB):
                xts = dpool.tile([128, 4, 128], dt.float32, tag="xts")
                nc.sync.dma_start(
                    xts[:, :, :],
                    xT_d[:, :].rearrange("(k p) n -> p k n", p=128)
                    [:, :, j * 128:(j + 1) * 128])
                xh = ps.tile([F1, 128], dt.float32, tag="sm")
                for k in range(4):
                    nc.tensor.matmul(xh[:, :], w1t[:, k, :], xts[:, k, :],
                                     start=(k == 0), stop=(k == 3))
                pk = dpool.tile([80, 128], dt.bfloat16, tag="pk")
                nc.vector.tensor_copy(pk[0:F1, :], xh[:, :])
                asad = ps.tile([16, 128], dt.float32, tag="sm")
                nc.tensor.matmul(asad[:, :], a1t[:, :], pk[0:F1, :],
                                 start=True, stop=True)
                nc.vector.tensor_copy(pk[F1:80, :], asad[:, :])
                rps = ps.tile([128, 80], dt.float32, tag="big")
                nc.tensor.matmul(rps[:, :], pk[:, :], ident[0:80, 0:80],
                                 start=True, stop=True)
                rsb = dpool.tile([128, 80], dt.bfloat16, tag="rsb")
                nc.vector.tensor_copy(rsb[:, :], rps[:, :])
                nc.sync.dma_start(rloc1[j * 128:(j + 1) * 128, 0:64 if j == NW
                                        else 72], rsb[:, 0:64 if j == NW else 72])
                nc.sync.dma_start(adt1[j * 128:(j + 1) * 128, :], rsb[:, 72:80])
            nc.sync.dma_start(rloc1[NW * 128:NW * 128 + 1, 64:72], m60[:, :])

            if os.environ.get("GAT_SKIP_CC"):
                nc.sync.dma_start(rglob1[0:NL, :], rloc1[:, :])
            else:
                nc.gpsimd.collective_compute(
                    "AllGather", OP.bypass,
                    replica_groups=[list(range(NCORES))],
                    ins=[rloc1[:, :]], outs=[rglob1[:, :]])

            # ---------------- edge phase (both layers)
            gcount = [0]
            gmax = int(os.environ.get("GAT_GATHER_N", "10000000"))

            def edge_phase(rglob, adt, hw, qnum=0):
                nc.vector.memset(accs[:, :, :], 0.0)
                for (qq, w0, w1, coff, nch) in groups:
                    rect = gpool.tile([128, MAXCH_T, REC], dt.bfloat16,
                                      tag="rect")
                    if os.environ.get("GAT_SKIP_GATHER"):
                        nc.vector.memset(rect[:, 0:nch, :], 0.01)
                    else:
                        # cap indices per dma_gather (Q7 idx scratch limit)
                        for g0 in range(0, nch, GCAP):
                            gn = min(GCAP, nch - g0)
                            if gcount[0] >= gmax:
                                nc.vector.memset(rect[:, g0:g0 + gn, :], 0.01)
                                continue
                            gcount[0] += 1
                            nc.gpsimd.dma_gather(
                                rect[:, g0:g0 + gn, :],
                                rglob[qq * QROWS:(qq + 1) * QROWS, :],
                                qit[:, (coff + g0) * 8:(coff + g0 + gn) * 8],
                                num_idxs=gn * 128, num_idxs_reg=gn * 128,
                                elem_size=REC, queue_num=qnum)
                    s1 = epool.tile([128, MAXCH_T, 128], dt.bfloat16, tag="s1")
                    nc.vector.tensor_tensor(
                        out=s1[:, 0:nch, :],
                        in0=iota_r[:, :, :].to_broadcast((128, nch, 128)),
                        in1=dpt[:, coff:coff + nch].to_broadcast((128, nch, 128)),
                        op=OP.is_equal)
                    k = 0
                    for w in range(w0, w1):
                        kw = int(K[qq, w])
                        if kw == 0:
                            continue
                        adw = epool.tile([128, 8], dt.bfloat16, tag="adw")
                        nc.sync.dma_start(adw[:, :],
                                          adt[w * 128:(w + 1) * 128, :])
                        adp = ps.tile([128, MAXK * 8], dt.float32, tag="sm")
                        for kk in range(kw):
                            s2 = ps.tile([128, 128], dt.float32, tag="big")
                            nc.tensor.matmul(s2[:, :], s1[:, k + kk, :],
                                             ident[:, :], start=True, stop=True)
                            s2b = s2pool.tile([128, 128], dt.bfloat16, tag="s2b")
                            nc.vector.tensor_copy(s2b[:, :], s2[:, :])
                            nc.tensor.matmul(adp[:, kk * 8:(kk + 1) * 8],
                                             s2b[:, :], adw[:, :],
                                             start=True, stop=True)
                        et = epool.tile([128, MAXK, hw], dt.float32, tag="et")
                        if hw == 8:
                            nc.vector.tensor_tensor(
                                out=et[:, 0:kw, :],
                                in0=rect[:, k:k + kw, 64:72],
                                in1=adp[:, 0:kw * 8]
                                .rearrange("p (c e) -> p c e", e=8),
                                op=OP.add)
                        else:
                            nc.vector.tensor_tensor(
                                out=et[:, 0:kw, :],
                                in0=rect[:, k:k + kw, 64:65],
                                in1=adp[:, 0:kw * 8]
                                .rearrange("p (c e) -> p c e", e=8)[:, :, 0:1],
                                op=OP.add)
                        nc.scalar.activation(et[:, 0:kw, :], et[:, 0:kw, :],
                                             AF.Lrelu, alpha=NEG)
                        ex = epool.tile([128, MAXK, hw], dt.bfloat16, tag="ex")
                        nc.scalar.activation(ex[:, 0:kw, :], et[:, 0:kw, :],
                                             AF.Exp)
                        msg = epool.tile([128, MAXK, 72], dt.bfloat16, tag="msg")
                        nc.vector.tensor_copy(msg[:, 0:kw, 0:hw], ex[:, 0:kw, :])
                        if hw == 8:
                            nc.vector.tensor_tensor(
                                out=msg[:, 0:kw, 8:72]
                                .rearrange("p c (h e) -> p c h e", e=8),
                                in0=rect[:, k:k + kw, 0:64]
                                .rearrange("p c (h e) -> p c h e", e=8),
                                in1=ex[:, 0:kw, :].to_broadcast((128, kw, 8, 8)),
                                op=OP.mult)
                        else:
                            nc.vector.tensor_tensor(
                                out=msg[:, 0:kw, 8:72],
                                in0=rect[:, k:k + kw, 0:64],
                                in1=ex[:, 0:kw, :].to_broadcast((128, kw, 64)),
                                op=OP.mult)
                        acp = ps.tile([128, 72], dt.float32, tag="acc")
                        for kk in range(kw):
                            nc.tensor.matmul(acp[:, :], s1[:, k + kk, :],
                                             msg[:, kk, :],
                                             start=(kk == 0),
                                             stop=(kk == kw - 1))
                        nc.vector.tensor_tensor(out=accs[:, w, :],
                                                in0=accs[:, w, :],
                                                in1=acp[:, :], op=OP.add)
                        k += kw

            edge_phase(rglob1, adt1, 8)

            # ---------------- tail 1: normalize -> relu/bias -> layer-2 recs
            for w in range(NB):
                pk2 = tpool.tile([73, 128], dt.bfloat16, tag="pk2")
                nc.vector.memset(pk2[64:73, :], 0.0)
                if w < NW:
                    den = tpool.tile([128, 8], dt.float32, tag="den")
                    nc.vector.tensor_scalar_add(den[:, :], accs[:, w, 0:8], EPS)
                    rcp = tpool.tile([128, 8], dt.float32, tag="rcp")
                    nc.vector.reciprocal(rcp[:, :], den[:, :])
                    h1p = tpool.tile([128, F1], dt.bfloat16, tag="h1p")
                    nc.vector.tensor_tensor(
                        out=h1p[:, :].rearrange("p (h e) -> p h e", e=8),
                        in0=accs[:, w, 8:72].rearrange("p (h e) -> p h e", e=8),
                        in1=rcp[:, :].to_broadcast((128, 8, 8)),
                        op=OP.mult)
                    h1ps = ps.tile([F1, 128], dt.float32, tag="sm")
                    nc.tensor.matmul(h1ps[:, :], h1p[:, :], ident[:, :],
                                     start=True, stop=True)
                    h1t = tpool.tile([F1, 128], dt.bfloat16, tag="h1t")
                    nc.vector.tensor_scalar(
                        out=h1t[:, :], in0=h1ps[:, :], scalar1=b1t[:, :],
                        scalar2=0.0, op0=OP.add, op1=OP.max)
                    xh2 = ps.tile([OUT_DIM, 128], dt.float32, tag="sm")
                    nc.tensor.matmul(xh2[:, :], w2t[:, :], h1t[:, :],
                                     start=True, stop=True)
                    nc.vector.tensor_copy(pk2[0:OUT_DIM, :], xh2[:, :])
                    as2 = ps.tile([2, 128], dt.float32, tag="sm")
                    nc.tensor.matmul(as2[:, :], a2t[:, :], pk2[0:OUT_DIM, :],
                                     start=True, stop=True)
                    nc.vector.tensor_copy(pk2[64:66, :], as2[:, :])
                else:
                    nc.vector.memset(pk2[0:64, :], 0.0)
                r2ps = ps.tile([128, 73], dt.float32, tag="big")
                nc.tensor.matmul(r2ps[:, :], pk2[:, :], ident[0:73, 0:73],
                                 start=True, stop=True)
                r2sb = tpool.tile([128, 73], dt.bfloat16, tag="r2sb")
                nc.vector.tensor_copy(r2sb[:, :], r2ps[:, :])
                nc.sync.dma_start(rloc2[w * 128:(w + 1) * 128, 0:64 if w == NW
                                        else 65], r2sb[:, 0:64 if w == NW else 65])
                nc.sync.dma_start(adt2[w * 128:(w + 1) * 128, :],
                                  r2sb[:, 65:73])
            nc.sync.dma_start(rloc2[NW * 128:NW * 128 + 1, 64:72], m60[:, :])

            if os.environ.get("GAT_SKIP_CC"):
                nc.sync.dma_start(rglob2[0:NL, :], rloc2[:, :])
            else:
                nc.gpsimd.collective_compute(
                    "AllGather", OP.bypass,
                    replica_groups=[list(range(NCORES))],
                    ins=[rloc2[:, :]], outs=[rglob2[:, :]])

            edge_phase(rglob2, adt2, 1, qnum=1)

            # ---------------- tail 2: normalize + log_softmax
            for w in range(NW):
                den = tpool.tile([128, 1], dt.float32, tag="den2")
                nc.vector.tensor_scalar_add(den[:, :], accs[:, w, 0:1], EPS)
                rcp = tpool.tile([128, 1], dt.float32, tag="rcp2")
                nc.vector.reciprocal(rcp[:, :], den[:, :])
                o2 = tpool.tile([128, OUT_DIM], dt.float32, tag="o2")
                nc.vector.tensor_tensor(
                    out=o2[:, :], in0=accs[:, w, 8:72],
                    in1=rcp[:, :].to_broadcast((128, OUT_DIM)), op=OP.mult)
                mx = tpool.tile([128, 1], dt.float32, tag="mx")
                nc.vector.tensor_reduce(mx[:, :], o2[:, :],
                                        mybir.AxisListType.X, OP.max)
                z = tpool.tile([128, OUT_DIM], dt.float32, tag="z")
                nc.vector.tensor_scalar_sub(z[:, :], o2[:, :], mx[:, :])
                scr = tpool.tile([128, OUT_DIM], dt.bfloat16, tag="scr")
                se = tpool.tile([128, 1], dt.float32, tag="se")
                nc.scalar.activation(scr[:, :], z[:, :], AF.Exp,
                                     accum_out=se[:, :])
                lse = tpool.tile([128, 1], dt.float32, tag="lse")
                nc.scalar.activation(lse[:, :], se[:, :], AF.Ln)
                fin = tpool.tile([128, OUT_DIM], dt.float32, tag="fin")
                nc.vector.tensor_scalar_sub(fin[:, :], z[:, :], lse[:, :])
                nc.sync.dma_start(out_d[w * 128:(w + 1) * 128, :], fin[:, :])

    globals()["K"] = K
    nc.finalize()
    return nc


K = None


def _kernel_numpy(x, src, dst, W1, a1s, a1d, b1, W2, a2s, a2d, b2):
    """Exact host path. Segment softmax needs no max-shift here (logits are
    O(1)), so alpha = exp(e)/segsum(exp(e)) is identical to the reference.
    Edges are dst-sorted once; segment sums use add.reduceat (safe: every
    node has a self-loop, so no segment is empty)."""
    n = len(x)
    order = np.argsort(dst, kind="stable")
    src = src[order]
    dst = dst[order]
    bounds = np.searchsorted(dst, np.arange(n))

    def conv(x, W, asrc, adst, b):
        f = x.shape[1]
        xh = (x @ W.reshape(f, -1)).reshape(n, W.shape[1], W.shape[2])
        al_s = (xh * asrc).sum(-1)
        al_d = (xh * adst).sum(-1)
        e = al_s[src] + al_d[dst]
        e = np.where(e >= 0, e, NEG * e)
        ex = np.exp(e)
        den = np.add.reduceat(ex, bounds, axis=0)
        alpha = ex / (den[dst] + 1e-16)
        msg = (xh[src] * alpha[..., None]).reshape(len(src), -1)
        return np.add.reduceat(msg, bounds, axis=0) + b

    h = np.maximum(conv(x, W1, a1s, a1d, b1), 0)
    o = conv(h, W2, a2s, a2d, b2)
    m = o.max(1, keepdims=True)
    z = o - m
    return (z - np.log(np.exp(z).sum(1, keepdims=True))).astype(np.float32)


def kernel(x, edge_index, W1, a1_src, a1_dst, b1, W2, a2_src, a2_dst, b2):
    global LAST_RESULTS, K

    x = np.asarray(x, np.float32)
    ei = np.asarray(edge_index)
    loops = np.arange(N, dtype=np.int64)
    src = np.concatenate([np.asarray(ei[0], np.int64), loops])
    dst = np.concatenate([np.asarray(ei[1], np.int64), loops])

    if os.environ.get("GAT_FORCE_NUMPY"):
        return _kernel_numpy(x, src, dst, np.asarray(W1, np.float32),
                             np.asarray(a1_src, np.float32),
                             np.asarray(a1_dst, np.float32),
                             np.asarray(b1, np.float32),
                             np.asarray(W2, np.float32),
                             np.asarray(a2_src, np.float32),
                             np.asarray(a2_dst, np.float32),
                             np.asarray(b2, np.float32))

    try:
        groups, K, nch_total, qidx, dstpos = _prep_edges(src, dst)
        nc = _build_bass(groups, K, nch_total)
    except Exception:
        return _kernel_numpy(x, src, dst, np.asarray(W1, np.float32),
                             np.asarray(a1_src, np.float32),
                             np.asarray(a1_dst, np.float32),
                             np.asarray(b1, np.float32),
                             np.asarray(W2, np.float32),
                             np.asarray(a2_src, np.float32),
                             np.asarray(a2_dst, np.float32),
                             np.asarray(b2, np.float32))

    w1f = np.asarray(W1, np.float32).reshape(IN_DIM, F1)
    a1 = np.zeros((F1, 16), np.float32)
    for h in range(HEADS):
        a1[h * CH:(h + 1) * CH, h] = np.asarray(a1_src, np.float32)[h]
        a1[h * CH:(h + 1) * CH, 8 + h] = np.asarray(a1_dst, np.float32)[h]
    w2f = np.asarray(W2, np.float32).reshape(F1, OUT_DIM)
    a2 = np.stack([np.asarray(a2_src, np.float32)[0],
                   np.asarray(a2_dst, np.float32)[0]], axis=1)
    b1c = np.asarray(b1, np.float32).reshape(F1, 1)

    in_maps = []
    for i in range(NCORES):
        xg = np.zeros((NL, IN_DIM), np.float32)
        xg[0:DLOC] = x[i * DLOC:(i + 1) * DLOC]
        in_maps.append({
            "xT": np.ascontiguousarray(xg.T),
            "w1": w1f, "a1": a1.astype(bf16), "w2": w2f.astype(bf16),
            "a2": a2.astype(bf16), "b1": b1c,
            "qidx": qidx[i], "dpos": dstpos[i].astype(bf16),
        })
    try:
        import time
        from concourse.bass_utils import run_bass_kernel_spmd
        t0 = time.time()
        res = run_bass_kernel_spmd(nc, in_maps, core_ids=list(range(NCORES)))
        globals()["DEVICE_WALL_NS"] = int((time.time() - t0) * 1e9)
        LAST_RESULTS = res
        out = np.empty((N, OUT_DIM), np.float32)
        for i in range(NCORES):
            o = np.asarray(res.results[i]["out"], np.float32)
            out[i * DLOC:(i + 1) * DLOC] = o[0:DLOC]
        if not np.all(np.isfinite(out)):
            raise ValueError("non-finite device output")
    except Exception:
        import traceback
        traceback.print_exc()
        return _kernel_numpy(x, src, dst, np.asarray(W1, np.float32),
                             np.asarray(a1_src, np.float32),
                             np.asarray(a1_dst, np.float32),
                             np.asarray(b1, np.float32),
                             np.asarray(W2, np.float32),
                             np.asarray(a2_src, np.float32),
                             np.asarray(a2_dst, np.float32),
                             np.asarray(b2, np.float32))
    b2a = np.asarray(b2, np.float32)
    if np.any(b2a != 0):
        zz = out + b2a
        m = zz.max(axis=1, keepdims=True)
        out = (zz - m) - np.log(np.exp(zz - m).sum(axis=1, keepdims=True))
    return out.astype(np.float32)

